# revision 56
# baseline (speedup 1.0000x reference)
"""DGN agent (2-layer graph attention) Trainium2 Bass kernel, v5 (bf16).

Dataflow per 128-row group (4 batches x 32 agents, block-diagonal):
  - Activations feature-major [128 feat, rows]; all matmuls bf16, fp32
    PSUM accumulation.
  - Scores ROW-major s[i,j]: exp on ACT; mask-mult + segmented row-sum
    + broadcast-normalize + 32x32 stream transpose on DVE.
  - Out-projection refactor: ow.T(att@v) = (v@ow).T attT. u = v@ow runs
    as group matmuls (lhsT = v_fm slice); h' = u.T @ attT lands
    feature-major with per-partition relu+bias.
  - Wave-pipelined emission (2-deep): wave w interleaves layer 2 of
    row-tile w-1 chunk-by-chunk with encoder+layer 1 of row-tile w
    (layer 1 staggered one chunk behind the encoder), so each engine's
    in-order FIFO always holds ready work from an independent stage.
    This hides the per-chunk softmax-chain latency, keeps the PE HAM
    clock up, fills drain bubbles, and keeps the fill/drain ramps short.
  - Each group-quad's ph matmuls + h' drain are emitted inline one
    chunk behind its softmax chain (not in a per-stage tail), smearing
    the ACT drain load across the wave instead of lumping it at wave
    boundaries.
  - Drains alternate ACT/DVE (q,v on ACT; k alternating; u-copy
    alternating) to balance the two drain engines.
"""

import numpy as np

import concourse.bass as bass
import concourse.mybir as mybir
import concourse.tile as tile
from concourse import bacc
from concourse.bass_utils import run_bass_kernel_spmd

F32 = mybir.dt.float32
BF16 = mybir.dt.bfloat16
AX = mybir.AxisListType
OP = mybir.AluOpType
AF = mybir.ActivationFunctionType

B, N, DIN, H, A = 4096, 32, 256, 128, 32
NCORES = 8
BC = B // NCORES          # batches per core
R = BC * N                # rows per core (16384)

# wblob column offsets (bf16 [128, 1312])
WOFF = {"enc": 0, "a1_q": 256, "a1_k": 384, "a1_v": 512, "a1_o": 640,
        "a2_q": 768, "a2_k": 896, "a2_v": 1024, "a2_o": 1152, "qw": 1280}
WCOLS = 1312
# bblob column index (fp32 [128, 10])
BOFF = {"enc": 0, "a1_q": 1, "a1_k": 2, "a1_v": 3, "a1_o": 4,
        "a2_q": 5, "a2_k": 6, "a2_v": 7, "a2_o": 8, "qb": 9}


def build_program(n_rows, rt=2048, n_cores=NCORES, sbufs=4):
    assert n_rows % rt == 0 and rt % 512 == 0
    n_rt = n_rows // rt
    gpt = rt // 128           # groups per row tile
    ngg = gpt // 4            # psum-bank quads per row tile
    ck = rt // 512            # 512-col chunks per row tile

    nc = bacc.Bacc("TRN2", target_bir_lowering=False, debug=False,
                   num_devices=n_cores)

    xt_d = nc.dram_tensor("xt", [DIN, n_rows], BF16, kind="ExternalInput")
    mc_d = nc.dram_tensor("metc", [128, n_rows // 128, 128], BF16,
                          kind="ExternalInput")
    wb_d = nc.dram_tensor("wblob", [128, WCOLS], BF16, kind="ExternalInput")
    bb_d = nc.dram_tensor("bblob", [128, len(BOFF)], F32,
                          kind="ExternalInput")
    y_d = nc.dram_tensor("y", [A, n_rows], F32, kind="ExternalOutput")

    with tile.TileContext(nc) as tc:
        with (
            tc.tile_pool(name="singles", bufs=1) as singles,
            tc.tile_pool(name="xt", bufs=3) as xt_pool,
            tc.tile_pool(name="met", bufs=4) as met_pool,
            tc.tile_pool(name="acts", bufs=8) as act_pool,
            tc.tile_pool(name="qkv", bufs=3) as qkv_pool,
            tc.tile_pool(name="sm", bufs=sbufs) as sm_pool,
            tc.tile_pool(name="out", bufs=2) as out_pool,
            tc.tile_pool(name="pproj", bufs=3, space="PSUM") as pproj,
            tc.tile_pool(name="psc", bufs=2, space="PSUM") as psc_pool,
            tc.tile_pool(name="patt", bufs=3, space="PSUM") as patt_pool,
        ):
            wb = singles.tile([128, WCOLS], BF16, tag="wb")
            nc.sync.dma_start(out=wb, in_=wb_d.ap())
            bb = singles.tile([128, len(BOFF)], F32, tag="bb")
            nc.sync.dma_start(out=bb, in_=bb_d.ap())

            def W(nm, w=H):
                return wb[:, WOFF[nm]:WOFF[nm] + w]

            def Bi(nm):
                return bb[:, BOFF[nm]:BOFF[nm] + 1]

            def drain_act(out, in_, bnm):
                nc.scalar.activation(out=out, in_=in_, func=AF.Relu,
                                     bias=Bi(bnm), scale=1.0)

            def drain_dve(out, in_, bnm):
                nc.vector.tensor_scalar(out=out, in0=in_, scalar1=Bi(bnm),
                                        scalar2=0.0, op0=OP.add, op1=OP.max)

            def make_s0(irt):
                """DMA + encoder for row tile irt as per-chunk steps so
                they interleave into the wave's chunk loop; returns
                (chunk_fn, (act, mc))."""
                r0 = irt * rt
                xt_sb = xt_pool.tile([128, 2, rt], BF16, tag="xt")
                nc.sync.dma_start(
                    out=xt_sb,
                    in_=xt_d.ap().rearrange("(c k) r -> k c r", c=2)
                    [:, :, r0:r0 + rt])
                mc_sb = met_pool.tile([128, gpt, 128], BF16, tag="mc")
                nc.sync.dma_start(
                    out=mc_sb,
                    in_=mc_d.ap()[:, r0 // 128:r0 // 128 + gpt, :])
                act = act_pool.tile([128, rt], BF16, tag="act")

                def chunk(c):
                    sl = bass.ts(c, 512)
                    ps = pproj.tile([128, 512], F32, tag="proj")
                    nc.tensor.matmul(ps, W("enc", 256).rearrange(
                        "p (c h) -> p c h", c=2)[:, 0, :],
                        xt_sb[:, 0, sl], start=True, stop=False)
                    nc.tensor.matmul(ps, W("enc", 256).rearrange(
                        "p (c h) -> p c h", c=2)[:, 1, :],
                        xt_sb[:, 1, sl], start=False, stop=True)
                    if c % 2 == 0:
                        drain_act(act[:, sl], ps, "enc")
                    else:
                        drain_dve(act[:, sl], ps, "enc")

                return chunk, (act, mc_sb)

            def make_layer(lname, act, mc_sb):
                """Returns (chunk_fn, tail_fn) emitting one attention
                layer over `act`; chunk_fn(c) emits projections +
                score/u matmuls + softmax chain for 512-col chunk c,
                tail_fn() emits the ph matmuls + output drains."""
                q_sb = qkv_pool.tile([128, rt], BF16, tag="q")
                k_sb = qkv_pool.tile([128, rt], BF16, tag="k")
                v_sb = qkv_pool.tile([128, rt], BF16, tag="v")
                nact = act_pool.tile([128, rt], BF16, tag="act")
                attTs, u_sbs = [], []

                def chunk(c):
                    sl = bass.ts(c, 512)
                    kdr = drain_dve if c % 2 == 0 else drain_act
                    for dst, pnm, dr in ((q_sb, "q", drain_act),
                                         (k_sb, "k", kdr),
                                         (v_sb, "v", drain_act)):
                        ps = pproj.tile([128, 512], F32, tag="proj")
                        nc.tensor.matmul(ps, W(f"{lname}_{pnm}"),
                                         act[:, sl], start=True, stop=True)
                        dr(dst[:, sl], ps, f"{lname}_{pnm}")

                    g0 = c * 4
                    sc = psc_pool.tile([128, 4, 128], F32, tag="sc")
                    for gi in range(4):
                        gsl = bass.ts(g0 + gi, 128)
                        nc.tensor.matmul(sc[:, gi, :], q_sb[:, gsl],
                                         k_sb[:, gsl], start=True,
                                         stop=True)
                    pu = patt_pool.tile([128, 4, 128], F32, tag="uh")
                    for gi in range(4):
                        gsl = bass.ts(g0 + gi, 128)
                        nc.tensor.matmul(pu[:, gi, :], v_sb[:, gsl],
                                         W(f"{lname}_o"), start=True,
                                         stop=True)
                    eraw = sm_pool.tile([128, 4, 128], BF16, tag="eraw")
                    nc.scalar.activation(out=eraw, in_=sc, func=AF.Exp,
                                         bias=0.0, scale=1.0)
                    e_sb = sm_pool.tile([128, 4, 128], BF16, tag="e")
                    nc.vector.tensor_tensor(
                        out=e_sb, in0=eraw,
                        in1=mc_sb[:, g0:g0 + 4, :], op=OP.mult)
                    rs = sm_pool.tile([128, 4], F32, tag="rs")
                    nc.vector.tensor_reduce(out=rs, in_=e_sb, axis=AX.X,
                                            op=OP.add)
                    rr = sm_pool.tile([128, 4], F32, tag="rr")
                    nc.vector.reciprocal(out=rr, in_=rs)
                    att = sm_pool.tile([128, 4, 128], BF16, tag="att")
                    nc.vector.scalar_tensor_tensor(
                        out=att, in0=e_sb, scalar=1.0,
                        in1=rr[:, :, None].broadcast_to([128, 4, 128]),
                        op0=OP.bypass, op1=OP.mult)
                    attT = sm_pool.tile([128, 4, 128], BF16, tag="attT",
                                        bufs=2 * ngg + 2)
                    nc.vector.transpose(
                        out=attT.rearrange("p g w -> p (g w)"),
                        in_=att.rearrange("p g w -> p (g w)"))
                    u_sb = sm_pool.tile([128, 4, 128], BF16, tag="u_sb",
                                        bufs=2 * ngg + 2)
                    if c % 2 == 0:
                        nc.scalar.copy(out=u_sb, in_=pu)
                    else:
                        nc.vector.tensor_copy(out=u_sb, in_=pu)
                    attTs.append(attT)
                    u_sbs.append(u_sb)

                def phgg(gg):
                    ph = patt_pool.tile([128, 4, 128], F32, tag="uh")
                    for gi in range(4):
                        nc.tensor.matmul(ph[:, gi, :],
                                         u_sbs[gg][:, gi, :],
                                         attTs[gg][:, gi, :],
                                         start=True, stop=True)
                    nc.scalar.activation(
                        out=nact[:, bass.ts(gg, 512)],
                        in_=ph.rearrange("p g w -> p (g w)"),
                        func=AF.Relu, bias=Bi(f"{lname}_o"), scale=1.0)
                    return nact

                return chunk, phgg

            def emit_final(irt, act):
                r0 = irt * rt
                for c0 in range(0, ck, 4):
                    nq = min(4, ck - c0)
                    m = A * nq
                    ps = pproj.tile([128, 512], F32, tag="proj")
                    for ci in range(nq):
                        c = c0 + ci
                        nc.tensor.matmul(ps[A * ci:A * (ci + 1), :],
                                         W("qw", A), act[:, bass.ts(c, 512)],
                                         start=True, stop=True,
                                         skip_group_check=True,
                                         tile_position=(0, A * ci))
                    o_sb = out_pool.tile([128, 512], F32, tag="o")
                    nc.vector.tensor_scalar(out=o_sb[:m, :], in0=ps[:m, :],
                                            scalar1=Bi("qb")[:m, :],
                                            scalar2=None, op0=OP.add)
                    for ci in range(nq):
                        c = c0 + ci
                        nc.sync.dma_start(
                            out=y_d.ap()[:, r0 + 512 * c:r0 + 512 * (c + 1)],
                            in_=o_sb[A * ci:A * (ci + 1), :])

            # ---- wave-pipelined emission over row tiles ----------
            # Wave w: L2 of row-tile w-1 interleaves chunk-by-chunk with
            # enc+L1 of row-tile w (L1 staggered one chunk behind enc —
            # make_layer only needs tile handles, so the pair shares a
            # wave).  2-deep pipeline: shorter fill/drain ramps.
            acts1, acts2, mcs = {}, {}, {}
            for w in range(n_rt + 1):
                s2 = s1 = s0 = None
                if 0 <= w - 1 < n_rt:
                    s2 = make_layer("a2", acts1.pop(w - 1), mcs[w - 1])
                if w < n_rt:
                    s0 = make_s0(w)
                    mcs[w] = s0[1][1]
                    s1 = make_layer("a1", s0[1][0], s0[1][1])
                for c in range(ck):
                    if s2 is not None:
                        s2[0](c)
                        if c >= 1:
                            s2[1](c - 1)
                    if s0 is not None:
                        s0[0](c)
                    if s1 is not None and c >= 1:
                        s1[0](c - 1)
                        if c >= 2:
                            s1[1](c - 2)
                if s2 is not None:
                    acts2[w - 1] = s2[1](ck - 1)
                if s1 is not None:
                    s1[0](ck - 1)
                    if ck >= 2:
                        s1[1](ck - 2)
                    acts1[w] = s1[1](ck - 1)
                if s2 is not None:
                    emit_final(w - 1, acts2.pop(w - 1))
                    mcs.pop(w - 1, None)

    nc.compile()
    return nc


def _bf16(a):
    import ml_dtypes
    return np.asarray(a, np.float32).astype(ml_dtypes.bfloat16)


def pack_weights(inputs):
    """Build the bf16 weight blob [128, WCOLS] and fp32 bias blob."""
    wb = np.zeros((128, WCOLS), np.float32)
    ew = np.asarray(inputs["enc_w"], np.float32)          # [256, 128]
    wb[:, 0:128] = ew[:128]
    wb[:, 128:256] = ew[128:]
    for l in ("a1", "a2"):
        for p in ("q", "k", "v", "o"):
            wb[:, WOFF[f"{l}_{p}"]:WOFF[f"{l}_{p}"] + H] = np.asarray(
                inputs[f"{l}_{p}w"], np.float32)
    wb[:, WOFF["qw"]:WOFF["qw"] + A] = np.asarray(inputs["q_w"], np.float32)
    bbl = np.zeros((128, len(BOFF)), np.float32)
    bbl[:, 0] = inputs["enc_b"]
    for l in ("a1", "a2"):
        for i, p in enumerate(("q", "k", "v", "o")):
            bbl[:, BOFF[f"{l}_{p}"]] = inputs[f"{l}_{p}b"]
    bbl[:, BOFF["qb"]] = np.tile(np.asarray(inputs["q_b"], np.float32), 4)
    return _bf16(wb), bbl


def prep_inputs_core(x_c, mask_c):
    """Per-core: x -> bf16 [DIN, rows]; mask -> (1-met) bf16 block tiles."""
    rows = x_c.shape[0] * N
    xt = _bf16(np.ascontiguousarray(
        np.asarray(x_c, np.float32).reshape(rows, DIN).T))
    ng = x_c.shape[0] // 4
    m4 = np.asarray(mask_c, np.float32).reshape(ng, 4, N, N)
    met = np.zeros((ng, 128, 128), np.float32)
    for b in range(4):
        met[:, 32 * b:32 * b + 32, 32 * b:32 * b + 32] = m4[:, b]
    mc = np.ascontiguousarray(met.transpose(1, 0, 2))      # [128, G, 128]
    return xt, _bf16(mc)


_CACHE = {}


def build_in_maps(inputs):
    x, mask = inputs["x"], inputs["mask"]
    wb, bbl = pack_weights(inputs)
    in_maps = []
    for c in range(NCORES):
        xt, mc = prep_inputs_core(x[c * BC:(c + 1) * BC],
                                  mask[c * BC:(c + 1) * BC])
        in_maps.append({"xt": xt, "metc": mc, "wblob": wb, "bblob": bbl})
    return in_maps


def kernel(**inputs):
    if "nc" not in _CACHE:
        _CACHE["nc"] = build_program(R)
    res = run_bass_kernel_spmd(_CACHE["nc"], build_in_maps(inputs),
                               core_ids=list(range(NCORES)))
    outs = [r["y"].T.reshape(BC, N, A) for r in res.results]
    return np.concatenate(outs, axis=0).astype(np.float32)
